# revision 34
# baseline (speedup 1.0000x reference)
"""Trainium2 Bass kernel for an 8-step complex DMD recurrence.

Math (matching the reference):
  Ag[0]=A[0], Ag[p]=A[8-p] (p>=1), all complex [M,M].
  uc window w_t (len 8) starts as the real inputs x_0..x_7; each step
    u2_t = sum_p Ag[p] @ w_t[p]   (complex, [B,M])
  then the window slides.  Output = Re([u2_1..u2_8]) as [B, 8, M].

Strategy (8 NeuronCores on a 4x2 grid: 4 m-shards x 2 batch-halves):
  * core c owns m rows [256*(c%4), +256) and batch half c//4 (128 of
    B=256).  All matmul inputs are fp16 (PSUM accumulates fp32): halves
    every DMA + collective byte at the same PE rate; recurrence error
    stays ~1e-3 (budget 2e-2).
  * one PSUM bank per step t: bank_t = [128, 512] fp32 with quadrants
    (mt, re|im) of 128 batch cols each.  Every term accumulates in PSUM
    via matmuls only:
      x-terms:        re += Ar@x,        im += Ai@x
      recurrent j:    re += Ar@ur - Ai@ui  (wnai = -Ai host-prepared)
                      im += Ai@ur + Ar@ui
    so step t ends with a single PSUM->SBUF fp16 copy.
  * after each step an AllGather within each 4-core m-group (same batch
    half) shares the [256,128] re/im slice: 512KB gathered => ~28us in
    the collective cost model (vs 41us for an 8-way fp32->fp16 gather
    and 67us for the 8-way fp32 one).  The 7 gathers serialize through
    the recurrence, so program order is "eager scatter": when gather_j
    lands, first the chain term (u2_j -> bank_{j+1}), then u2_j's
    contributions to banks j+2..8 and spare x-terms act as PE filler
    under the next gather.
"""

import numpy as np

B, L, M = 256, 8, 1024
N_CORES = 8
MG = 4           # m-shard groups (cores per gather group)
BH = 2           # batch halves
MB = M // MG     # 256 m rows per core
NB = B // BH     # 128 batch cols per core
NT = M // 128    # 8 contraction tiles of 128
MT = MB // 128   # 2 output row tiles per core
P_STEPS = 8

_CACHE = {}


def _build_program():
    import concourse.bacc as bacc
    import concourse.mybir as mybir
    import concourse.tile as tile
    from concourse.bass import ts

    dt = mybir.dt
    f16 = dt.float16
    f32 = dt.float32

    nc = bacc.Bacc("TRN2", target_bir_lowering=False, debug=False,
                   num_devices=N_CORES)

    # Inputs (per core). Partition-major layouts prepared on the host:
    #   war/wai/wnai: [p, pos, nt, mt, m] -> [128, 8*8*2*128]
    #       (AgT slices for this core's 256 m rows; wnai = -wai)
    #   xw:           [p, q, nt, b] -> [128, 8*8*128]  (this batch half)
    WCOLS = L * NT * MT * 128
    war = nc.dram_tensor("war", [128, WCOLS], f16, kind="ExternalInput")
    wai = nc.dram_tensor("wai", [128, WCOLS], f16, kind="ExternalInput")
    xw = nc.dram_tensor("xw", [128, L * NT * NB], f16, kind="ExternalInput")
    out = nc.dram_tensor("out", [P_STEPS, MB, NB], f32, kind="ExternalOutput")

    # Collective buffers (one pair per gathered step t=1..7).
    # ci rows: (mt, h, p) of this core's slice; co rows: (g, mt, h, p).
    cc_in = [nc.dram_tensor(f"cc_in{t}", [MT * 2 * 128, NB], f16)
             for t in range(1, 8)]
    cc_out = [
        nc.dram_tensor(f"cc_out{t}", [MG * MT * 2 * 128, NB], f16)
        for t in range(1, 8)
    ]

    # gather within each batch-half's 4 m-shard cores
    rg = [[0, 1, 2, 3], [4, 5, 6, 7]]

    # ---- program-order plan ----------------------------------------
    plan = []
    for pos in range(8):
        plan.append(("x", 1, pos))
    plan.append(("c", 1))
    # wnai = -wai is built on the idle DVE during gather_1 rather than
    # DMA'd: its 3.5MB otherwise interleaves with the war/wai/xw stream
    # on the DMA engine and pushes the first gather out by ~11us.  Only
    # pos 1..7 are ever used by recurrent terms (pos = 8-t+j >= 1).
    plan.append(("w",))
    for t in (2, 3):
        for pos in range(9 - t):
            plan.append(("x", t, pos))
    xfill = {1: 4, 2: 5, 3: 6, 4: 7, 5: 8}
    # keep-warm dummy matmuls per gather window: real filler runs short
    # in late windows, and a cold PE restarts at the low p-state (197ns
    # per matmul instead of 53ns) right when the chain burst arrives.
    dummies = {}
    for j in range(1, 8):
        plan.append(("rs", j + 1, j))     # chain term: needs gather_j
        plan.append(("c", j + 1))
        for t in range(j + 2, 9):         # far contributions: filler
            plan.append(("r", t, j))
        if j in xfill:
            t = xfill[j]
            for pos in range(9 - t):
                plan.append(("x", t, pos))
        if dummies.get(j):
            plan.append(("d", dummies[j]))

    # per-(bank, quadrant) totals for start/stop flags; quadrants are
    # symmetric: x-term adds NT per quadrant, r-term adds 2*NT.
    # bank 8's imaginary half is never consumed (no gather after step 8,
    # output is Re only) so its h=1 matmuls are skipped entirely.
    mm_total = {t: 0 for t in range(1, 9)}
    for op in plan:
        if op[0] == "x":
            mm_total[op[1]] += NT
        elif op[0] == "r":
            mm_total[op[1]] += 2 * NT
    mm_seen = {(t, mt, h): 0 for t in range(1, 9) for mt in range(MT)
               for h in (0, 1)}

    with tile.TileContext(nc) as tc:
        with (
            tc.tile_pool(name="a", bufs=1) as apool,
            tc.tile_pool(name="xp", bufs=8) as xpool,
            tc.tile_pool(name="sl", bufs=7) as slpool,
            tc.tile_pool(name="stg", bufs=4) as stpool,
            tc.tile_pool(name="bk", bufs=8, space="PSUM") as bkpool,
        ):
            t_war = apool.tile([128, WCOLS], f16, tag="war")
            t_wai = apool.tile([128, WCOLS], f16, tag="wai")
            t_wnai = apool.tile([128, WCOLS], f16, tag="wnai")

            def wtile(which, pos, nt, mt):
                t = (t_war, t_wai, t_wnai)[which]
                return t[:, ts((pos * NT + nt) * MT + mt, 128)]

            banks = {
                t: bkpool.tile([128, 512], f32, tag="bank", name=f"bank{t}")
                for t in range(1, 9)
            }
            xtiles = {}
            slots = {}

            def load_pos(pos):
                # spread across engine queues -> parallel DMA rings
                sl = ts(pos, NT * MT * 128)
                nc.sync.dma_start(t_war[:, sl], war[:, sl])
                nc.scalar.dma_start(t_wai[:, sl], wai[:, sl])
                xt = xpool.tile([128, NT * NB], f16, tag="xt")
                nc.gpsimd.dma_start(xt[:], xw[:, ts(pos, NT * NB)])
                xtiles[pos] = xt

            def mm(bank_t, mt, h, w, rhs):
                if bank_t == 8 and h == 1:
                    return
                first = all(
                    mm_seen[bank_t, m2, h2] == 0
                    for m2 in range(MT) for h2 in (0, 1)
                )
                mm_seen[bank_t, mt, h] += 1
                last = mm_seen[bank_t, mt, h] == mm_total[bank_t]
                nc.tensor.matmul(
                    banks[bank_t][:, ts(mt * 2 + h, 128)], w, rhs,
                    start=first,
                    stop=last,
                    skip_group_check=True,
                )

            def xterm(t, pos):
                xt = xtiles[pos + t - 1]
                for nt in range(NT):
                    rhs = xt[:, ts(nt, NB)]
                    for mt in range(MT):
                        mm(t, mt, 0, wtile(0, pos, nt, mt), rhs)
                        mm(t, mt, 1, wtile(1, pos, nt, mt), rhs)

            def rterm(t, j, split=False):
                # split=True (chain term): emit every left-half (re)
                # matmul first so the re half closes early and its copy +
                # staging DMA overlap the right-half matmuls.
                pos = 8 - t + j

                def quads(nt):
                    ch = slots[j][nt // MT]  # per-chunk tile: fine deps
                    sub = nt % MT
                    ur = ch[:, ts(sub * 2, 128)]
                    ui = ch[:, ts(sub * 2 + 1, 128)]
                    return ur, ui

                if split:
                    for nt in range(NT):
                        ur, ui = quads(nt)
                        for mt in range(MT):
                            mm(t, mt, 0, wtile(0, pos, nt, mt), ur)
                            mm(t, mt, 0, wtile(2, pos, nt, mt), ui)
                    for nt in range(NT):
                        ur, ui = quads(nt)
                        for mt in range(MT):
                            mm(t, mt, 1, wtile(1, pos, nt, mt), ur)
                            mm(t, mt, 1, wtile(0, pos, nt, mt), ui)
                else:
                    for nt in range(NT):
                        ur, ui = quads(nt)
                        for mt in range(MT):
                            mm(t, mt, 0, wtile(0, pos, nt, mt), ur)
                            mm(t, mt, 0, wtile(2, pos, nt, mt), ui)
                            mm(t, mt, 1, wtile(1, pos, nt, mt), ur)
                            mm(t, mt, 1, wtile(0, pos, nt, mt), ui)

            def combine(t):
                if t < 8:
                    # per-half copy + staging DMA: the re half closes
                    # before the chain's im matmuls finish, so its leg
                    # overlaps them (critical path: im leg -> collective)
                    stg_h = stpool.tile([128, 512], f16, tag="stg_h")
                    ci, co = cc_in[t - 1], cc_out[t - 1]
                    for h in (0, 1):
                        src = banks[t][:].rearrange(
                            "p (mt h b) -> p mt h b", mt=MT, h=2, b=NB
                        )[:, :, h, :]
                        dsth = stg_h[:].rearrange(
                            "p (mt h b) -> p mt h b", mt=MT, h=2, b=NB
                        )[:, :, h, :]
                        nc.vector.tensor_copy(dsth, src)
                        nc.sync.dma_start(
                            ci[:].rearrange("(mt h p) b -> p mt h b", mt=MT,
                                            h=2, p=128)[:, :, h, :],
                            dsth,
                        )
                    nc.gpsimd.collective_compute(
                        "AllGather", mybir.AluOpType.bypass,
                        replica_groups=rg, ins=[ci[:]], outs=[co[:]],
                    )
                    # one slot TILE per group chunk: chain matmuls on
                    # ktiles of chunk g start as soon as chunk g lands
                    chunks = []
                    qs = (nc.sync, nc.scalar, nc.gpsimd, nc.sync)
                    for g in range(MG):
                        ch = slpool.tile([128, MT * 2 * NB], f16,
                                         tag=f"slot{g}")
                        src = co[g * MT * 256:(g + 1) * MT * 256, :].rearrange(
                            "(mt h p) b -> p mt h b", mt=MT, h=2, p=128)
                        dst = ch[:].rearrange(
                            "p (mt h b) -> p mt h b", mt=MT, h=2, b=NB)
                        qs[g].dma_start(dst, src)
                        chunks.append(ch)
                    slots[t] = chunks
                # this core's slice of Re(u2_t) -> output row t-1
                stg_o = stpool.tile([128, MT * NB], f32, tag="stg_o")
                for mt in range(MT):
                    nc.vector.tensor_copy(
                        stg_o[:, ts(mt, NB)], banks[t][:, ts(mt * 2, 128)])
                nc.sync.dma_start(
                    out[t - 1].rearrange("(mt p) b -> p mt b", mt=MT, p=128),
                    stg_o[:].rearrange("p (mt b) -> p mt b", mt=MT, b=NB),
                )

            def dummy(n):
                # PE keep-warm: overwrite retired bank_1, never read again
                for i in range(n):
                    nc.tensor.matmul(
                        banks[1][:, 0:128], wtile(0, 0, i % NT, 0),
                        xtiles[0][:, ts(i % NT, NB)],
                        start=True, stop=True, skip_group_check=True,
                    )

            # ---- emit in plan order ----
            for pos in range(8):
                load_pos(pos)
            for op in plan:
                if op[0] == "x":
                    xterm(op[1], op[2])
                elif op[0] == "rs":
                    rterm(op[1], op[2], split=True)
                elif op[0] == "r":
                    rterm(op[1], op[2])
                elif op[0] == "d":
                    dummy(op[1])
                elif op[0] == "w":
                    for pos in range(7, 0, -1):
                        sl = ts(pos, NT * MT * 128)
                        nc.vector.tensor_scalar_mul(
                            t_wnai[:, sl], t_wai[:, sl], -1.0)
                else:
                    combine(op[1])

    nc.compile()
    return nc


def _get_runner():
    if "runner" in _CACHE:
        return _CACHE["runner"]

    import jax
    from jax.sharding import Mesh, PartitionSpec
    from jax.experimental.shard_map import shard_map
    import concourse.mybir as mybir
    from concourse import bass2jax

    nc = _build_program()
    bass2jax.install_neuronx_cc_hook()
    partition_name = nc.partition_id_tensor.name if nc.partition_id_tensor else None
    in_names, out_names, out_avals, zero_outs = [], [], [], []
    for alloc in nc.m.functions[0].allocations:
        if not isinstance(alloc, mybir.MemoryLocationSet):
            continue
        name = alloc.memorylocations[0].name
        if alloc.kind == "ExternalInput":
            if name != partition_name:
                in_names.append(name)
        elif alloc.kind == "ExternalOutput":
            out_names.append(name)
            shape = tuple(alloc.tensor_shape)
            dtype = mybir.dt.np(alloc.dtype)
            out_avals.append(jax.core.ShapedArray(shape, dtype))
            zero_outs.append(np.zeros(shape, dtype))
    n_params = len(in_names)
    n_outs = len(out_avals)
    all_in = in_names + out_names + ([partition_name] if partition_name else [])
    donate = tuple(range(n_params, n_params + n_outs))

    def _body(*args):
        operands = list(args)
        if partition_name is not None:
            operands.append(bass2jax.partition_id_tensor())
        return tuple(
            bass2jax._bass_exec_p.bind(
                *operands,
                out_avals=tuple(out_avals),
                in_names=tuple(all_in),
                out_names=tuple(out_names),
                lowering_input_output_aliases=(),
                sim_require_finite=True,
                sim_require_nnan=True,
                nc=nc,
            )
        )

    devices = jax.devices()[:N_CORES]
    mesh = Mesh(np.asarray(devices), ("core",))
    sharded = jax.jit(
        shard_map(
            _body, mesh=mesh,
            in_specs=(PartitionSpec("core"),) * (n_params + n_outs),
            out_specs=(PartitionSpec("core"),) * n_outs,
            check_rep=False,
        ),
        donate_argnums=donate,
        keep_unused=True,
    )
    runner = {
        "sharded": sharded,
        "in_names": in_names,
        "out_names": out_names,
        "out_avals": out_avals,
        "zero_outs": zero_outs,
        "mesh": mesh,
    }
    _CACHE["runner"] = runner
    return runner


def prepare_inputs(x, A_real, A_imag):
    """Host-side reorder/transpose into the kernel's DMA-friendly layouts."""
    x = np.asarray(x, dtype=np.float32)
    A_real = np.asarray(A_real, dtype=np.float32)
    A_imag = np.asarray(A_imag, dtype=np.float32)
    idx = np.concatenate([[0], np.arange(L - 1, 0, -1)]).astype(np.int64)
    Agr = A_real[idx]  # [k, m, n]
    Agi = A_imag[idx]
    # AgT: [k, n, m]; per m-block slice; partition-major [p, k, nt, mt, m]
    AgrT = np.ascontiguousarray(Agr.transpose(0, 2, 1))
    AgiT = np.ascontiguousarray(Agi.transpose(0, 2, 1))

    def wlayout(AT, mb):
        sl = AT[:, :, mb * MB:(mb + 1) * MB]  # [k, n, 256]
        w = (sl.reshape(L, NT, 128, MT, 128)
             .transpose(2, 0, 1, 3, 4).reshape(128, -1))
        return np.ascontiguousarray(w.astype(np.float16))

    wars = [wlayout(AgrT, mb) for mb in range(MG)]
    wais = [wlayout(AgiT, mb) for mb in range(MG)]
    # x: [b, q, m] -> per batch half: [p, q, nt, b]
    xws = []
    for bh in range(BH):
        xs = x[bh * NB:(bh + 1) * NB]  # [128, L, M]
        xt = xs.transpose(1, 2, 0).reshape(L, NT, 128, NB)
        xws.append(np.ascontiguousarray(
            xt.transpose(2, 0, 1, 3).reshape(128, -1).astype(np.float16)))
    return wars, wais, xws


def make_in_maps(x, A_real, A_imag):
    wars, wais, xws = prepare_inputs(x, A_real, A_imag)
    maps = []
    for c in range(N_CORES):
        bh, mb = c // MG, c % MG
        maps.append({"war": wars[mb], "wai": wais[mb], "xw": xws[bh]})
    return maps


def kernel(x, A_real, A_imag, predict_length):
    P = int(predict_length)
    if P != P_STEPS:  # pragma: no cover - reference always uses 8
        return _numpy_fallback(x, A_real, A_imag, P)

    import jax

    runner = _get_runner()
    in_maps = make_in_maps(x, A_real, A_imag)
    concat_in = [
        np.concatenate([m[n] for m in in_maps], axis=0) for n in runner["in_names"]
    ]
    czeros = [
        np.zeros((N_CORES * z.shape[0], *z.shape[1:]), z.dtype)
        for z in runner["zero_outs"]
    ]
    out_arrs = runner["sharded"](*concat_in, *czeros)
    jax.block_until_ready(out_arrs)
    o = np.asarray(out_arrs[0]).reshape(N_CORES, P_STEPS, MB, NB)
    # core c=(bh, mb): o[c, t] = [m256, b128] -> full[b, t, m]
    full = np.empty((B, P_STEPS, M), np.float32)
    for c in range(N_CORES):
        bh, mb = c // MG, c % MG
        full[bh * NB:(bh + 1) * NB, :, mb * MB:(mb + 1) * MB] = (
            o[c].transpose(2, 0, 1))
    return np.ascontiguousarray(full)


def _numpy_fallback(x, A_real, A_imag, P):
    A = (np.asarray(A_real) + 1j * np.asarray(A_imag)).astype(np.complex64)
    idx = np.concatenate([[0], np.arange(L - 1, 0, -1)]).astype(np.int64)
    Ag = A[idx]
    uc = np.asarray(x).astype(np.complex64)
    for _ in range(P):
        u2 = np.einsum("kmn,bkn->bm", Ag, uc)
        uc = np.concatenate([uc[:, 1:], u2[:, None]], axis=1)
    return np.real(uc).astype(np.float32)


# revision 36
# speedup vs baseline: 3.9729x; 3.9729x over previous
"""Trainium2 Bass kernel for an 8-step complex DMD recurrence.

Math (matching the reference):
  Ag[0]=A[0], Ag[p]=A[8-p] (p>=1), all complex [M,M].
  uc window w_t (len 8) starts as the real inputs x_0..x_7; each step
    u2_t = sum_p Ag[p] @ w_t[p]   (complex, [B,M])
  then the window slides.  Output = Re([u2_1..u2_8]) as [B, 8, M].

Strategy (8 NeuronCores on a 4x2 grid: 4 m-shards x 2 batch-halves):
  * core c owns m rows [256*(c%4), +256) and batch half c//4 (128 of
    B=256).  All matmul inputs are fp16 (PSUM accumulates fp32): halves
    every DMA + collective byte at the same PE rate; recurrence error
    stays ~1e-3 (budget 2e-2).
  * one PSUM bank per step t: bank_t = [128, 512] fp32 with quadrants
    (mt, re|im) of 128 batch cols each.  Every term accumulates in PSUM
    via matmuls only:
      x-terms:        re += Ar@x,        im += Ai@x
      recurrent j:    re += Ar@ur - Ai@ui  (wnai = -Ai host-prepared)
                      im += Ai@ur + Ar@ui
    so step t ends with a single PSUM->SBUF fp16 copy.
  * after each step an AllGather within each 4-core m-group (same batch
    half) shares the [256,128] re/im slice: 512KB gathered => ~28us in
    the collective cost model (vs 41us for an 8-way fp32->fp16 gather
    and 67us for the 8-way fp32 one).  The 7 gathers serialize through
    the recurrence, so program order is "eager scatter": when gather_j
    lands, first the chain term (u2_j -> bank_{j+1}), then u2_j's
    contributions to banks j+2..8 and spare x-terms act as PE filler
    under the next gather.
"""

import numpy as np

B, L, M = 256, 8, 1024
N_CORES = 8
MG = 4           # m-shard groups (cores per gather group)
BH = 2           # batch halves
MB = M // MG     # 256 m rows per core
NB = B // BH     # 128 batch cols per core
NT = M // 128    # 8 contraction tiles of 128
MT = MB // 128   # 2 output row tiles per core
P_STEPS = 8

_CACHE = {}


def _build_program():
    import concourse.bacc as bacc
    import concourse.mybir as mybir
    import concourse.tile as tile
    from concourse.bass import ts

    dt = mybir.dt
    f16 = dt.float16
    f32 = dt.float32

    nc = bacc.Bacc("TRN2", target_bir_lowering=False, debug=False,
                   num_devices=N_CORES)

    # Inputs (per core). Partition-major layouts prepared on the host:
    #   war/wai/wnai: [p, pos, nt, mt, m] -> [128, 8*8*2*128]
    #       (AgT slices for this core's 256 m rows; wnai = -wai)
    #   xw:           [p, q, nt, b] -> [128, 8*8*128]  (this batch half)
    WCOLS = L * NT * MT * 128
    war = nc.dram_tensor("war", [128, WCOLS], f16, kind="ExternalInput")
    wai = nc.dram_tensor("wai", [128, WCOLS], f16, kind="ExternalInput")
    xw = nc.dram_tensor("xw", [128, L * NT * NB], f16, kind="ExternalInput")
    out = nc.dram_tensor("out", [P_STEPS, MB, NB], f32, kind="ExternalOutput")

    # Collective buffers (one pair per gathered step t=1..7).
    # ci rows: (mt, h, p) of this core's slice; co rows: (g, mt, h, p).
    cc_in = [nc.dram_tensor(f"cc_in{t}", [MT * 2 * 128, NB], f16)
             for t in range(1, 8)]
    cc_out = [
        nc.dram_tensor(f"cc_out{t}", [MG * MT * 2 * 128, NB], f16)
        for t in range(1, 8)
    ]

    # gather within each batch-half's 4 m-shard cores
    rg = [[0, 1, 2, 3], [4, 5, 6, 7]]

    # ---- program-order plan ----------------------------------------
    plan = []
    for pos in range(8):
        plan.append(("x", 1, pos))
    plan.append(("c", 1))
    # wnai = -wai is built on the idle DVE during gather_1 rather than
    # DMA'd: its 3.5MB otherwise interleaves with the war/wai/xw stream
    # on the DMA engine and pushes the first gather out by ~11us.  Only
    # pos 1..7 are ever used by recurrent terms (pos = 8-t+j >= 1).
    plan.append(("w",))
    for t in (2, 3):
        for pos in range(9 - t):
            plan.append(("x", t, pos))
    xfill = {1: 4, 2: 5, 3: 6, 4: 7, 5: 8}
    # keep-warm dummy matmuls per gather window: real filler runs short
    # in late windows, and a cold PE restarts at the low p-state (197ns
    # per matmul instead of 53ns) right when the chain burst arrives.
    dummies = {}
    for j in range(1, 8):
        plan.append(("rs", j + 1, j))     # chain term: needs gather_j
        plan.append(("c", j + 1))
        for t in range(j + 2, 9):         # far contributions: filler
            plan.append(("r", t, j))
        if j in xfill:
            t = xfill[j]
            for pos in range(9 - t):
                plan.append(("x", t, pos))
        if dummies.get(j):
            plan.append(("d", dummies[j]))

    # per-(bank, quadrant) totals for start/stop flags; quadrants are
    # symmetric: x-term adds NT per quadrant, r-term adds 2*NT.
    # bank 8's imaginary half is never consumed (no gather after step 8,
    # output is Re only) so its h=1 matmuls are skipped entirely.
    mm_total = {t: 0 for t in range(1, 9)}
    for op in plan:
        if op[0] == "x":
            mm_total[op[1]] += NT
        elif op[0] == "r":
            mm_total[op[1]] += 2 * NT
    mm_seen = {(t, mt, h): 0 for t in range(1, 9) for mt in range(MT)
               for h in (0, 1)}

    with tile.TileContext(nc) as tc:
        with (
            tc.tile_pool(name="a", bufs=1) as apool,
            tc.tile_pool(name="xp", bufs=8) as xpool,
            tc.tile_pool(name="sl", bufs=7) as slpool,
            tc.tile_pool(name="stg", bufs=4) as stpool,
            tc.tile_pool(name="bk", bufs=8, space="PSUM") as bkpool,
        ):
            t_war = apool.tile([128, WCOLS], f16, tag="war")
            t_wai = apool.tile([128, WCOLS], f16, tag="wai")
            t_wnai = apool.tile([128, WCOLS], f16, tag="wnai")

            def wtile(which, pos, nt, mt):
                t = (t_war, t_wai, t_wnai)[which]
                return t[:, ts((pos * NT + nt) * MT + mt, 128)]

            banks = {
                t: bkpool.tile([128, 512], f32, tag="bank", name=f"bank{t}")
                for t in range(1, 9)
            }
            xtiles = {}
            slots = {}

            def load_pos(pos):
                # spread across engine queues -> parallel DMA rings
                sl = ts(pos, NT * MT * 128)
                nc.sync.dma_start(t_war[:, sl], war[:, sl])
                nc.scalar.dma_start(t_wai[:, sl], wai[:, sl])
                xt = xpool.tile([128, NT * NB], f16, tag="xt")
                nc.gpsimd.dma_start(xt[:], xw[:, ts(pos, NT * NB)])
                xtiles[pos] = xt

            def mm(bank_t, mt, h, w, rhs):
                if bank_t == 8 and h == 1:
                    return
                first = all(
                    mm_seen[bank_t, m2, h2] == 0
                    for m2 in range(MT) for h2 in (0, 1)
                )
                mm_seen[bank_t, mt, h] += 1
                last = mm_seen[bank_t, mt, h] == mm_total[bank_t]
                nc.tensor.matmul(
                    banks[bank_t][:, ts(mt * 2 + h, 128)], w, rhs,
                    start=first,
                    stop=last,
                    skip_group_check=True,
                )

            def xterm(t, pos):
                xt = xtiles[pos + t - 1]
                for nt in range(NT):
                    rhs = xt[:, ts(nt, NB)]
                    for mt in range(MT):
                        mm(t, mt, 0, wtile(0, pos, nt, mt), rhs)
                        mm(t, mt, 1, wtile(1, pos, nt, mt), rhs)

            def rterm(t, j, split=False):
                # split=True (chain term): emit every left-half (re)
                # matmul first so the re half closes early and its copy +
                # staging DMA overlap the right-half matmuls.
                pos = 8 - t + j

                def quads(nt):
                    ch = slots[j][nt // MT]  # per-chunk tile: fine deps
                    sub = nt % MT
                    ur = ch[:, ts(sub * 2, 128)]
                    ui = ch[:, ts(sub * 2 + 1, 128)]
                    return ur, ui

                if split:
                    for nt in range(NT):
                        ur, ui = quads(nt)
                        for mt in range(MT):
                            mm(t, mt, 0, wtile(0, pos, nt, mt), ur)
                            mm(t, mt, 0, wtile(2, pos, nt, mt), ui)
                    for nt in range(NT):
                        ur, ui = quads(nt)
                        for mt in range(MT):
                            mm(t, mt, 1, wtile(1, pos, nt, mt), ur)
                            mm(t, mt, 1, wtile(0, pos, nt, mt), ui)
                else:
                    for nt in range(NT):
                        ur, ui = quads(nt)
                        for mt in range(MT):
                            mm(t, mt, 0, wtile(0, pos, nt, mt), ur)
                            mm(t, mt, 0, wtile(2, pos, nt, mt), ui)
                            mm(t, mt, 1, wtile(1, pos, nt, mt), ur)
                            mm(t, mt, 1, wtile(0, pos, nt, mt), ui)

            def combine(t):
                if t < 8:
                    # per-half copy + staging DMA: the re half closes
                    # before the chain's im matmuls finish, so its leg
                    # overlaps them (critical path: im leg -> collective)
                    stg_h = stpool.tile([128, 512], f16, tag="stg_h")
                    ci, co = cc_in[t - 1], cc_out[t - 1]
                    for h in (0, 1):
                        src = banks[t][:].rearrange(
                            "p (mt h b) -> p mt h b", mt=MT, h=2, b=NB
                        )[:, :, h, :]
                        dsth = stg_h[:].rearrange(
                            "p (mt h b) -> p mt h b", mt=MT, h=2, b=NB
                        )[:, :, h, :]
                        nc.vector.tensor_copy(dsth, src)
                        nc.sync.dma_start(
                            ci[:].rearrange("(mt h p) b -> p mt h b", mt=MT,
                                            h=2, p=128)[:, :, h, :],
                            dsth,
                        )
                    nc.gpsimd.collective_compute(
                        "AllGather", mybir.AluOpType.bypass,
                        replica_groups=rg, ins=[ci[:]], outs=[co[:]],
                    )
                    # one slot TILE per group chunk: chain matmuls on
                    # ktiles of chunk g start as soon as chunk g lands
                    chunks = []
                    qs = (nc.sync, nc.scalar, nc.gpsimd, nc.sync)
                    for g in range(MG):
                        ch = slpool.tile([128, MT * 2 * NB], f16,
                                         tag=f"slot{g}")
                        src = co[g * MT * 256:(g + 1) * MT * 256, :].rearrange(
                            "(mt h p) b -> p mt h b", mt=MT, h=2, p=128)
                        dst = ch[:].rearrange(
                            "p (mt h b) -> p mt h b", mt=MT, h=2, b=NB)
                        qs[g].dma_start(dst, src)
                        chunks.append(ch)
                    slots[t] = chunks
                # this core's slice of Re(u2_t) -> output row t-1
                stg_o = stpool.tile([128, MT * NB], f32, tag="stg_o")
                for mt in range(MT):
                    nc.vector.tensor_copy(
                        stg_o[:, ts(mt, NB)], banks[t][:, ts(mt * 2, 128)])
                nc.sync.dma_start(
                    out[t - 1].rearrange("(mt p) b -> p mt b", mt=MT, p=128),
                    stg_o[:].rearrange("p (mt b) -> p mt b", mt=MT, b=NB),
                )

            def dummy(n):
                # PE keep-warm: overwrite retired bank_1, never read again
                for i in range(n):
                    nc.tensor.matmul(
                        banks[1][:, 0:128], wtile(0, 0, i % NT, 0),
                        xtiles[0][:, ts(i % NT, NB)],
                        start=True, stop=True, skip_group_check=True,
                    )

            # ---- emit in plan order ----
            for pos in range(8):
                load_pos(pos)
            for op in plan:
                if op[0] == "x":
                    xterm(op[1], op[2])
                elif op[0] == "rs":
                    rterm(op[1], op[2], split=True)
                elif op[0] == "r":
                    rterm(op[1], op[2])
                elif op[0] == "d":
                    dummy(op[1])
                elif op[0] == "w":
                    for pos in range(7, 0, -1):
                        sl = ts(pos, NT * MT * 128)
                        nc.vector.tensor_scalar_mul(
                            t_wnai[:, sl], t_wai[:, sl], -1.0)
                else:
                    combine(op[1])

    nc.compile()
    return nc


def _get_runner():
    if "runner" in _CACHE:
        return _CACHE["runner"]

    import jax
    from jax.sharding import Mesh, PartitionSpec
    from jax.experimental.shard_map import shard_map
    import concourse.mybir as mybir
    from concourse import bass2jax

    nc = _build_program()
    bass2jax.install_neuronx_cc_hook()
    partition_name = nc.partition_id_tensor.name if nc.partition_id_tensor else None
    in_names, out_names, out_avals, zero_outs = [], [], [], []
    for alloc in nc.m.functions[0].allocations:
        if not isinstance(alloc, mybir.MemoryLocationSet):
            continue
        name = alloc.memorylocations[0].name
        if alloc.kind == "ExternalInput":
            if name != partition_name:
                in_names.append(name)
        elif alloc.kind == "ExternalOutput":
            out_names.append(name)
            shape = tuple(alloc.tensor_shape)
            dtype = mybir.dt.np(alloc.dtype)
            out_avals.append(jax.core.ShapedArray(shape, dtype))
            zero_outs.append(np.zeros(shape, dtype))
    n_params = len(in_names)
    n_outs = len(out_avals)
    all_in = in_names + out_names + ([partition_name] if partition_name else [])
    donate = tuple(range(n_params, n_params + n_outs))

    def _body(*args):
        operands = list(args)
        if partition_name is not None:
            operands.append(bass2jax.partition_id_tensor())
        return tuple(
            bass2jax._bass_exec_p.bind(
                *operands,
                out_avals=tuple(out_avals),
                in_names=tuple(all_in),
                out_names=tuple(out_names),
                lowering_input_output_aliases=(),
                sim_require_finite=True,
                sim_require_nnan=True,
                nc=nc,
            )
        )

    devices = jax.devices()[:N_CORES]
    mesh = Mesh(np.asarray(devices), ("core",))
    sharded = jax.jit(
        shard_map(
            _body, mesh=mesh,
            in_specs=(PartitionSpec("core"),) * (n_params + n_outs),
            out_specs=(PartitionSpec("core"),) * n_outs,
            check_rep=False,
        ),
        donate_argnums=donate,
        keep_unused=True,
    )
    runner = {
        "sharded": sharded,
        "in_names": in_names,
        "out_names": out_names,
        "out_avals": out_avals,
        "zero_outs": zero_outs,
        "mesh": mesh,
    }
    _CACHE["runner"] = runner
    return runner


def prepare_inputs(x, A_real, A_imag):
    """Host-side reorder/transpose into the kernel's DMA-friendly layouts."""
    x = np.asarray(x, dtype=np.float32)
    A_real = np.asarray(A_real, dtype=np.float32)
    A_imag = np.asarray(A_imag, dtype=np.float32)
    idx = np.concatenate([[0], np.arange(L - 1, 0, -1)]).astype(np.int64)
    Agr = A_real[idx]  # [k, m, n]
    Agi = A_imag[idx]
    # AgT: [k, n, m]; per m-block slice; partition-major [p, k, nt, mt, m]
    AgrT = np.ascontiguousarray(Agr.transpose(0, 2, 1))
    AgiT = np.ascontiguousarray(Agi.transpose(0, 2, 1))

    def wlayout(AT, mb):
        sl = AT[:, :, mb * MB:(mb + 1) * MB]  # [k, n, 256]
        w = (sl.reshape(L, NT, 128, MT, 128)
             .transpose(2, 0, 1, 3, 4).reshape(128, -1))
        return np.ascontiguousarray(w.astype(np.float16))

    wars = [wlayout(AgrT, mb) for mb in range(MG)]
    wais = [wlayout(AgiT, mb) for mb in range(MG)]
    # x: [b, q, m] -> per batch half: [p, q, nt, b]
    xws = []
    for bh in range(BH):
        xs = x[bh * NB:(bh + 1) * NB]  # [128, L, M]
        xt = xs.transpose(1, 2, 0).reshape(L, NT, 128, NB)
        xws.append(np.ascontiguousarray(
            xt.transpose(2, 0, 1, 3).reshape(128, -1).astype(np.float16)))
    return wars, wais, xws


def make_in_maps(x, A_real, A_imag):
    wars, wais, xws = prepare_inputs(x, A_real, A_imag)
    maps = []
    for c in range(N_CORES):
        bh, mb = c // MG, c % MG
        maps.append({"war": wars[mb], "wai": wais[mb], "xw": xws[bh]})
    return maps


def kernel(x, A_real, A_imag, predict_length):
    P = int(predict_length)
    if P != P_STEPS:  # pragma: no cover - reference always uses 8
        return _numpy_fallback(x, A_real, A_imag, P)

    import jax

    runner = _get_runner()
    in_maps = make_in_maps(x, A_real, A_imag)
    concat_in = [
        np.concatenate([m[n] for m in in_maps], axis=0) for n in runner["in_names"]
    ]
    czeros = [
        np.zeros((N_CORES * z.shape[0], *z.shape[1:]), z.dtype)
        for z in runner["zero_outs"]
    ]
    out_arrs = runner["sharded"](*concat_in, *czeros)
    jax.block_until_ready(out_arrs)
    o = np.asarray(out_arrs[0]).reshape(N_CORES, P_STEPS, MB, NB)
    # core c=(bh, mb): o[c, t] = [m256, b128] -> full[b, t, m]
    full = np.empty((B, P_STEPS, M), np.float32)
    for c in range(N_CORES):
        bh, mb = c // MG, c % MG
        full[bh * NB:(bh + 1) * NB, :, mb * MB:(mb + 1) * MB] = (
            o[c].transpose(2, 0, 1))
    return np.ascontiguousarray(full)


def _numpy_fallback(x, A_real, A_imag, P):
    A = (np.asarray(A_real) + 1j * np.asarray(A_imag)).astype(np.complex64)
    idx = np.concatenate([[0], np.arange(L - 1, 0, -1)]).astype(np.int64)
    Ag = A[idx]
    uc = np.asarray(x).astype(np.complex64)
    for _ in range(P):
        u2 = np.einsum("kmn,bkn->bm", Ag, uc)
        uc = np.concatenate([uc[:, 1:], u2[:, None]], axis=1)
    return np.real(uc).astype(np.float32)
